# revision 41
# baseline (speedup 1.0000x reference)
"""GCN encoder (2-layer GCNConv + embedding lookup) on 8 trn2 NeuronCores.

Strategy (graph-parallel, per sharding hint):
  - Nodes are sharded across the 8 cores by id (12500 each), then renumbered
    into "slots": each core packs its nodes into G=112 groups of <=128 nodes
    with balanced per-section edge counts (bin-packing on host) and permutes
    groups into gather batches (GBS) with cross-core balanced sums, giving a
    uniform SPMD program.
  - norm folding: out = dis .* segsum(u[src]) + b with u = dis .* (x @ W),
    dis = deg^-1/2.
  - Layer 1 is type-factorized with HOST-prebuilt scatter matrices:
    S[c,t',g,b,w] = sum over edges into (core c, group g, slot w) of
    dis[src] where src type = 128*b + t' (self-loops included).  The device
    computes agg1T[d,w] = sum_b embW1_b^T @ S_gb (8 matmuls per group;
    embW1 = emb@W1 folded on host, bf16).  No per-edge device work for L1.
  - h1d = dis*relu(dis*agg1+b1) kept resident in SBUF (hd_all) and staged
    bf16 (128-wide rows = 256B gather elems); one AllGather -> full table.
  - Layer 2: per-batch edge gathers (dma_gather, int16 idx, midpoint-base
    A/B tables, batch-level shared-chunk padding ~+2%) feed one-hot matmul
    aggregation in PSUM; self-loops are added as a local diagonal matmul
    from hd_all (no gather), which also keeps the A/B sections symmetric
    across cores; W2 applied AFTER aggregation:
    out = dis * (segsum(h1d[src]) @ W2) + b2.
"""
import os
import sys

sys.path.insert(0, "/opt/trn_rl_repo")
import numpy as np
import ml_dtypes

BF16 = ml_dtypes.bfloat16

N_NODES = 100000
NCORE = 8
NPC = N_NODES // NCORE          # 12500 nodes per core
D1, D2 = 128, 64
NTYPES = 1000
G = 112                         # groups per core
W = 128                         # slots (nodes) per group
SLOTS = G * W                   # 14336 slots per core
TOTAL_SLOTS = NCORE * SLOTS     # 114688
BASE_A, BASE_B = 32768, 90112   # gather base rows (midpoint trick)
SPLIT_NODE = 4 * NPC            # src node < 50000 -> table A (cores 0-3)
PAD_DST = 999.0                 # dst_local sentinel -> zero one-hot row
SB = 4                          # groups per L1 S-matrix DMA load
GBS = [4] * 26 + [2] * 4        # groups per gather batch (tapered tail)
NBATCH = len(GBS)
POS0 = np.cumsum([0] + GBS)     # batch -> first group position


# ---------------------------------------------------------------- host prep
def _pack_core(nodes, degA, degB):
    """Greedy 2-d balanced packing of `nodes` into G groups of <=W nodes.
    Returns list of per-group node arrays."""
    a = degA[nodes].astype(np.float64)
    b = degB[nodes].astype(np.float64)
    order = np.argsort(-(a + b), kind="stable")
    tA = max(a.sum() / G, 1.0)
    tB = max(b.sum() / G, 1.0)
    sA = np.zeros(G)
    sB = np.zeros(G)
    cnt = np.zeros(G, np.int64)
    assign = np.empty(len(nodes), np.int64)
    for i in order:
        cost = np.maximum((sA + a[i]) / tA, (sB + b[i]) / tB)
        cost[cnt >= W] = np.inf
        g = int(np.argmin(cost))
        assign[i] = g
        sA[g] += a[i]
        sB[g] += b[i]
        cnt[g] += 1
    # refinement: push per-group section maxima down toward 8 chunks (1024)
    # by moving nodes out of the worst group into groups with slack.
    capA = capB = 1024
    for _ in range(20000):
        worstA = sA.max() / capA > sB.max() / capB
        s = sA if worstA else sB
        cap = capA if worstA else capB
        gsrc = int(np.argmax(s))
        if sA.max() <= capA and sB.max() <= capB:
            break
        need = s[gsrc] - cap
        members = np.where(assign == gsrc)[0]
        vals = (a if worstA else b)[members]
        cand = members[np.argsort(-(vals - need) * (vals >= need) - vals)]
        moved = False
        for i in cand[:30]:
            ai, bi = a[i], b[i]
            ok = (cnt < W) & (sA + ai <= capA) & (sB + bi <= capB)
            ok[gsrc] = False
            if ok.any():
                load = np.where(ok, np.maximum((sA + ai) / capA,
                                               (sB + bi) / capB), np.inf)
                gdst = int(np.argmin(load))
                assign[i] = gdst
                sA[gsrc] -= ai; sB[gsrc] -= bi; cnt[gsrc] -= 1
                sA[gdst] += ai; sB[gdst] += bi; cnt[gdst] += 1
                moved = True
                break
        if not moved:
            break
    groups = [nodes[assign == g] for g in range(G)]
    return groups


def preprocess(x_node_types, edge_index, emb, W1, b1, W2, b2):
    types = np.asarray(x_node_types).astype(np.int64)
    src = np.asarray(edge_index[0]).astype(np.int64)
    dst = np.asarray(edge_index[1]).astype(np.int64)
    loop = np.arange(N_NODES, dtype=np.int64)
    src_all = np.concatenate([src, loop])
    dst_all = np.concatenate([dst, loop])

    deg = np.bincount(dst_all, minlength=N_NODES).astype(np.float32)
    dis = (1.0 / np.sqrt(deg)).astype(np.float32)   # deg >= 1 (self loops)

    # self-loops are handled by a local diagonal add (no gather), so the
    # L2 edge layout (and the packer balancing it) uses only real edges.
    a_mask = src < SPLIT_NODE
    degA = np.bincount(dst[a_mask], minlength=N_NODES)
    degB = np.bincount(dst[~a_mask], minlength=N_NODES)

    # pack nodes -> slots (group order permuted per core so that the
    # gather batches [GBS] have balanced A/B section sums across cores)
    slot_of = np.full(N_NODES, -1, np.int64)
    slot2node = np.full((NCORE, SLOTS), -1, np.int64)
    for c in range(NCORE):
        nodes = np.arange(c * NPC, (c + 1) * NPC, dtype=np.int64)
        groups = _pack_core(nodes, degA, degB)
        ga = np.array([degA[gn].sum() for gn in groups], np.float64)
        gb = np.array([degB[gn].sum() for gn in groups], np.float64)
        perm = _form_batches(ga, gb)
        for p, g in enumerate(perm):
            gn = groups[g]
            s0 = p * W
            slot_of[gn] = c * SLOTS + s0 + np.arange(len(gn))
            slot2node[c, s0:s0 + len(gn)] = gn

    # per-edge data for L2 (real edges only; group id = batched position)
    e_srcslot = slot_of[src]
    e_dstslot = slot_of[dst]
    e_core = e_dstslot // SLOTS
    e_grp = (e_dstslot % SLOTS) // W
    e_dloc = e_dstslot % W
    e_sec = (src >= SPLIT_NODE).astype(np.int64)       # 0 = A, 1 = B
    e_bat = np.searchsorted(POS0, e_grp, side="right") - 1
    e_j = e_grp - POS0[e_bat]

    # order edges by (core, batch, sec, group-within-batch); rank in section
    MB = max(GBS)
    key = (((e_core * NBATCH + e_bat) * 2 + e_sec) * MB + e_j)
    order = np.argsort(key, kind="stable")
    nb2 = NCORE * NBATCH * 2 * MB
    cnt_key = np.bincount(key, minlength=nb2)
    offs = np.zeros(nb2 + 1, np.int64)
    np.cumsum(cnt_key, out=offs[1:])
    # length of (c, b, sec) section and group offsets within it
    cnt4 = cnt_key.reshape(NCORE, NBATCH, 2, MB)
    sec_len = cnt4.sum(axis=3)                     # [NCORE, NBATCH, 2]
    goff = np.zeros((NCORE, NBATCH, 2, MB + 1), np.int64)
    np.cumsum(cnt4, axis=3, out=goff[:, :, :, 1:])
    seckey = ((e_core * NBATCH + e_bat) * 2 + e_sec)
    sec_start = offs[(np.arange(nb2).reshape(-1, MB)[:, 0])]  # start per seckey
    q = np.arange(len(key)) - sec_start[seckey[order]]        # rank in section

    # per-batch chunk counts (>=1 trailing pad index guaranteed)
    NA = (sec_len[:, :, 0].max(axis=0) // 128 + 1).astype(np.int64)
    NB_ = (sec_len[:, :, 1].max(axis=0) // 128 + 1).astype(np.int64)
    # per-group chunk spans within the batch section, program-uniform
    lo_g = np.zeros((NCORE, NBATCH, 2, MB), np.int64)
    hi_g = np.zeros((NCORE, NBATCH, 2, MB), np.int64)
    lo_g[:] = goff[:, :, :, :MB] // 128
    hi_g[:] = np.maximum(goff[:, :, :, 1:] - 1, goff[:, :, :, :MB]) // 128 + 1
    spanA = [[(int(lo_g[:, b, 0, j].min()), int(hi_g[:, b, 0, j].max()))
              for j in range(GBS[b])] for b in range(NBATCH)]
    spanB = [[(int(lo_g[:, b, 1, j].min()), int(hi_g[:, b, 1, j].max()))
              for j in range(GBS[b])] for b in range(NBATCH)]
    # dstl column layout: per position p: spanA width then spanB width
    wA = [spanA[b][j][1] - spanA[b][j][0] for b in range(NBATCH)
          for j in range(GBS[b])]
    wB = [spanB[b][j][1] - spanB[b][j][0] for b in range(NBATCH)
          for j in range(GBS[b])]
    doff = np.zeros(G + 1, np.int64)
    np.cumsum(np.array(wA) + np.array(wB), out=doff[1:])
    NDSTL = int(doff[-1])

    # fill idx + dstl arrays
    aoff = np.zeros(NBATCH + 1, np.int64)
    np.cumsum(NA, out=aoff[1:])
    boff = np.zeros(NBATCH + 1, np.int64)
    np.cumsum(NB_, out=boff[1:])
    idxA_flat = np.zeros((NCORE, int(aoff[-1]) * 128), np.int16)
    idxB_flat = np.zeros((NCORE, int(boff[-1]) * 128), np.int16)
    dstl = np.full((NCORE, 128, NDSTL), PAD_DST, np.float32)

    oc = e_core[order]
    ob = e_bat[order]
    oj = e_j[order]
    osec = e_sec[order]
    oslot = e_srcslot[order]
    base = np.where(osec == 0, BASE_A, BASE_B)
    iv = oslot - base
    assert iv.min() >= -32768 and iv.max() <= 32767
    op_ = POS0[ob] + oj
    chunk = q // 128
    part = q % 128
    secoff = np.where(osec == 0, aoff[ob], boff[ob])
    mA = osec == 0
    mB = ~mA
    idxA_flat[oc[mA], (secoff[mA] + chunk[mA]) * 128 + part[mA]] = \
        iv[mA].astype(np.int16)
    idxB_flat[oc[mB], (secoff[mB] + chunk[mB]) * 128 + part[mB]] = \
        iv[mB].astype(np.int16)
    # dstl column: doff[p] + (chunk - lo) (+ wA for B section)
    lo_sel = np.where(mA,
                      np.array([[spanA[b][j][0] for j in range(GBS[b])] +
                                [0] * (MB - GBS[b]) for b in range(NBATCH)]
                               )[ob, oj],
                      np.array([[spanB[b][j][0] for j in range(GBS[b])] +
                                [0] * (MB - GBS[b]) for b in range(NBATCH)]
                               )[ob, oj])
    wa_arr = np.array(wA)[op_]
    col = doff[op_] + (chunk - lo_sel) + np.where(mA, 0, wa_arr)
    dstl[oc, part, col] = e_dloc[order].astype(np.float32)

    # wrapped int16 layout [128, n/16] (16-partition blocks replicated 8x)
    def wrap(vals):  # vals [..., n] -> [..., 128, n//16]
        n = vals.shape[-1]
        w = vals.reshape(*vals.shape[:-1], n // 16, 16)
        w = np.swapaxes(w, -1, -2)              # [..., 16, n//16]
        return np.tile(w, (1,) * (vals.ndim - 1) + (8, 1))

    gidxa_flat = np.ascontiguousarray(wrap(idxA_flat))  # [NCORE,128,sumNA*8]
    gidxb_flat = np.ascontiguousarray(wrap(idxB_flat))
    dstl_pc = np.ascontiguousarray(dstl)
    CMX = max(wA[p] + wB[p] for p in range(G))
    meta = dict(NA=[int(x) for x in NA], NB=[int(x) for x in NB_],
                spanA=spanA, spanB=spanB, doff=[int(x) for x in doff],
                wA=wA, wB=wB, CMX=CMX, NDSTL=NDSTL,
                aoff=[int(x) for x in aoff], boff=[int(x) for x in boff])

    # ---- L1 scatter matrices, host-prebuilt (dense, bf16-uploaded):
    # S[c, t', g, b, w] = sum dis[src] over edges (src type 128b+t') into
    # (core c, group g, slot w), SELF-LOOPS INCLUDED.
    # Device: agg1T = sum_b embW1_b^T @ S_gb.
    l_dstslot = slot_of[dst_all]
    l_core = l_dstslot // SLOTS
    l_grp = (l_dstslot % SLOTS) // W
    l_dloc = l_dstslot % W
    e_type = types[src_all]
    e_dis = dis[src_all].astype(np.float64)
    e_blk = e_type // 128
    e_tlow = e_type % 128
    s_up = np.zeros((NCORE, 128, G * 8 * W), np.float32)
    nbins = 128 * G * 8 * W
    for c in range(NCORE):
        m = l_core == c
        flat = (e_tlow[m] * G + l_grp[m]) * (8 * W) + e_blk[m] * W + l_dloc[m]
        s_up[c] = np.bincount(flat, weights=e_dis[m],
                              minlength=nbins).reshape(128, G * 8 * W)

    embW1f = (np.asarray(emb, np.float32) @ np.asarray(W1, np.float32)).astype(np.float32)
    embW1p = np.zeros((1024, D1), np.float32)
    embW1p[:NTYPES] = embW1f

    # quick consistency check (core 0): S reconstruction == direct
    _S0 = s_up[0].reshape(128, G, 8, W)
    _agg = np.einsum("tgbw,btd->gwd", _S0.astype(np.float64),
                     embW1p.reshape(8, 128, D1).astype(np.float64))
    _m0 = l_core == 0
    _agg2 = np.zeros((SLOTS, D1), np.float64)
    _s2 = (l_grp[_m0] * W + l_dloc[_m0]).astype(np.int64)
    np.add.at(_agg2, _s2, e_dis[_m0, None]
              * embW1p[e_type[_m0].astype(np.int64)])
    assert np.allclose(_agg.reshape(SLOTS, D1), _agg2, rtol=1e-5, atol=1e-5), \
        "S-matrix check failed"

    dis_slot = np.zeros((NCORE, SLOTS), np.float32)
    for c in range(NCORE):
        valid = slot2node[c] >= 0
        dis_slot[c, valid] = dis[slot2node[c, valid]]

    disb = np.ascontiguousarray(
        np.broadcast_to(
            dis_slot.reshape(NCORE, G, 1, W), (NCORE, G, 128, W)
        )
    ).astype(np.float32)
    dis_cols = np.ascontiguousarray(
        dis_slot.reshape(NCORE, G, W).transpose(0, 2, 1)
    ).astype(np.float32)                                   # [NCORE, 128, G]

    embW1 = embW1f
    emb8 = np.ascontiguousarray(
        embW1p.reshape(8, 128, D1).transpose(1, 0, 2).reshape(128, 8 * D1))
    iotac = np.tile(np.arange(W, dtype=np.float32)[None, :], (128, meta["CMX"]))
    b1c = np.asarray(b1, np.float32).reshape(128, 1)
    b2r = np.tile(np.asarray(b2, np.float32)[None, :], (128, 1))

    padded = (sum(meta["NA"]) + sum(meta["NB"])) * 128
    print(f"[preprocess] padded gather rows/core: {padded} "
          f"({padded / (len(src) / NCORE) - 1:+.1%})")

    return dict(
        meta=meta, dis=dis, slot2node=slot2node,
        dstl_pc=dstl_pc,
        gidxa_flat=gidxa_flat, gidxb_flat=gidxb_flat,
        disb=disb, dis_cols=dis_cols, dis_slot=dis_slot,
        embW1=embW1, b1c=b1c, b2r=b2r,
        w2=np.asarray(W2, np.float32),
        emb8=emb8, iotac=iotac, s_up=s_up,
    )


def _form_batches(ga, gb):
    """Order this core's G groups into NBATCH batches (sizes GBS) with
    balanced per-batch A/B section sums.  Returns the group permutation."""
    order = np.argsort(-(ga + gb), kind="stable")
    cap = np.array(GBS, np.float64)
    tA = ga.sum() / G
    tB = gb.sum() / G
    sA = np.zeros(NBATCH)
    sB = np.zeros(NBATCH)
    cnt = np.zeros(NBATCH, np.int64)
    batches = [[] for _ in range(NBATCH)]
    for g in order:
        load = np.maximum((sA + ga[g]) / (tA * cap), (sB + gb[g]) / (tB * cap))
        load[cnt >= cap] = np.inf
        k = int(np.argmin(load))
        batches[k].append(int(g))
        sA[k] += ga[g]
        sB[k] += gb[g]
        cnt[k] += 1
    # refinement: swap groups between batches to minimize the worst
    # normalized section sum (both sections count - NA/NB are ceil'd).
    for _ in range(3000):
        loadA = sA / (tA * cap)
        loadB = sB / (tB * cap)
        load = np.maximum(loadA, loadB)
        w = int(np.argmax(load))
        best = (0.0, None)
        lw = load[w]
        for gi in batches[w]:
            for o in range(NBATCH):
                if o == w:
                    continue
                for gj in batches[o]:
                    dA, dB = ga[gj] - ga[gi], gb[gj] - gb[gi]
                    nw = max((sA[w] + dA) / (tA * cap[w]),
                             (sB[w] + dB) / (tB * cap[w]))
                    no = max((sA[o] - dA) / (tA * cap[o]),
                             (sB[o] - dB) / (tB * cap[o]))
                    gain = lw - max(nw, no, load[o] if False else 0.0)
                    if max(nw, no) < lw - 1e-9 and gain > best[0]:
                        best = (gain, (gi, o, gj, dA, dB))
        if best[1] is None:
            break
        gi, o, gj, dA, dB = best[1]
        batches[w].remove(gi)
        batches[o].remove(gj)
        batches[w].append(gj)
        batches[o].append(gi)
        sA[w] += dA
        sB[w] += dB
        sA[o] -= dA
        sB[o] -= dB
    return [g for bt in batches for g in bt]


# ---------------------------------------------------------------- device
def build_program(meta):
    from concourse import bacc, mybir, tile

    NA, NB_ = meta["NA"], meta["NB"]
    spanA, spanB = meta["spanA"], meta["spanB"]
    doff, wA, wB = meta["doff"], meta["wA"], meta["wB"]
    aoff, boff = meta["aoff"], meta["boff"]
    CMX, NDSTL = meta["CMX"], meta["NDSTL"]
    NIDXA, NIDXB = aoff[-1] * 8, boff[-1] * 8
    f32, i16, bf16 = mybir.dt.float32, mybir.dt.int16, mybir.dt.bfloat16

    nc = bacc.Bacc(None, target_bir_lowering=False, num_devices=NCORE,
                   num_swdge_queues=4)
    emb8_in = nc.dram_tensor("emb8", [128, 8 * D1], bf16, kind="ExternalInput")
    s_in = nc.dram_tensor("s", [128, G * 8 * W], bf16, kind="ExternalInput")
    w2_in = nc.dram_tensor("w2", [D1, D2], bf16, kind="ExternalInput")
    gidxa_in = nc.dram_tensor("gidxa", [128, NIDXA], i16,
                              kind="ExternalInput")
    gidxb_in = nc.dram_tensor("gidxb", [128, NIDXB], i16,
                              kind="ExternalInput")
    dstlpc_in = nc.dram_tensor("dstlpc", [128, NDSTL], bf16,
                               kind="ExternalInput")
    disb_in = nc.dram_tensor("disb", [G, 128, W], bf16, kind="ExternalInput")
    discols_in = nc.dram_tensor("discols", [128, G], f32, kind="ExternalInput")
    iotac_in = nc.dram_tensor("iotac", [128, CMX * W], bf16,
                              kind="ExternalInput")
    ident_in = nc.dram_tensor("ident", [128, 128], bf16, kind="ExternalInput")
    b1c_in = nc.dram_tensor("b1c", [128, 1], f32, kind="ExternalInput")
    b2r_in = nc.dram_tensor("b2r", [128, D2], f32, kind="ExternalInput")
    out_ext = nc.dram_tensor("out", [SLOTS, D2], f32, kind="ExternalOutput")

    h1_stage = nc.dram_tensor("h1_stage", [SLOTS, D1], bf16)
    h1_full = nc.dram_tensor("h1_full", [TOTAL_SLOTS, D1], bf16, addr_space="Shared")

    RG = [list(range(NCORE))]
    Relu = mybir.ActivationFunctionType.Relu
    Copy = mybir.ActivationFunctionType.Copy

    def batch_gather(gat, idx_t, i0, nch, table, base_q):
        """One batch-section gather of nch*128 rows from `table`."""
        nc.gpsimd.dma_gather(
            out_ap=gat[:].rearrange("p (c d) -> p c d", d=D1),
            in_ap=table,
            idxs_ap=idx_t[:, i0 * 8:(i0 + nch) * 8],
            num_idxs=nch * 128, num_idxs_reg=nch * 128,
            elem_size=D1, single_packet=False, queue_num=base_q % 4,
        )

    ccw_in = nc.dram_tensor("ccw_in", [1, 128], f32)
    ccw_out = nc.dram_tensor("ccw_out", [NCORE, 128], f32, addr_space="Shared")

    with tile.TileContext(nc) as tc:
        with tc.tile_pool(name="cst", bufs=1) as cst:
            nc.gpsimd.collective_compute(
                "AllGather", mybir.AluOpType.bypass, replica_groups=RG,
                ins=[ccw_in[:]], outs=[ccw_out[:]],
            )
            w2_t = cst.tile([D1, D2], bf16)
            nc.sync.dma_start(out=w2_t[:], in_=w2_in[:])
            iotac_t = cst.tile([128, CMX * W], bf16)
            nc.sync.dma_start(out=iotac_t[:], in_=iotac_in[:])
            emb8_t = cst.tile([128, 8 * D1], bf16)
            nc.sync.dma_start(out=emb8_t[:], in_=emb8_in[:])
            ident_t = cst.tile([128, 128], bf16)
            nc.sync.dma_start(out=ident_t[:], in_=ident_in[:])
            b1c_t = cst.tile([128, 1], f32)
            nc.sync.dma_start(out=b1c_t[:], in_=b1c_in[:])
            b2r_t = cst.tile([128, D2], f32)
            nc.sync.dma_start(out=b2r_t[:], in_=b2r_in[:])
            discols_t = cst.tile([128, G], f32)
            nc.sync.dma_start(out=discols_t[:], in_=discols_in[:])
            gidxa_t = cst.tile([128, NIDXA], i16)
            nc.sync.dma_start(out=gidxa_t[:], in_=gidxa_in[:])
            gidxb_t = cst.tile([128, NIDXB], i16)
            nc.sync.dma_start(out=gidxb_t[:], in_=gidxb_in[:])
            dstl_t = cst.tile([128, NDSTL], bf16)
            nc.sync.dma_start(out=dstl_t[:], in_=dstlpc_in[:])
            # h1d kept resident for the local self-loop diagonal in L2
            hd_all = cst.tile([128, G * W], bf16)

            with tc.tile_pool(name="gat2", bufs=4) as gat2p, \
                 tc.tile_pool(name="ohp", bufs=4) as ohp, \
                 tc.tile_pool(name="sp", bufs=2) as spool, \
                 tc.tile_pool(name="sm", bufs=6) as sm, \
                 tc.tile_pool(name="hp", bufs=4) as hp, \
                 tc.tile_pool(name="op", bufs=3) as op, \
                 tc.tile_pool(name="ps1", bufs=4, space="PSUM") as ps1, \
                 tc.tile_pool(name="ps2", bufs=2, space="PSUM") as ps2:

                # ---- layer 1: 8 matmuls on host-built S + epilogue per group
                for blk in range(G // SB):
                    st = spool.tile([128, SB * 8 * W], bf16, tag="st")
                    nc.sync.dma_start(
                        out=st[:],
                        in_=s_in[:, blk * SB * 8 * W:(blk + 1) * SB * 8 * W])
                    dbt = sm.tile([128, SB * W], bf16, tag="dbt")
                    nc.scalar.dma_start(
                        out=dbt[:].rearrange("p (s w) -> p s w", w=W),
                        in_=disb_in[blk * SB:(blk + 1) * SB]
                        .rearrange("s p w -> p s w"))
                    hsb = op.tile([128, SB * D1], bf16, tag="hsb")
                    for j in range(SB):
                        g = blk * SB + j
                        aggT = ps1.tile([D1, W], f32, space="PSUM", tag="aggT")
                        for b in range(8):
                            nc.tensor.matmul(
                                out=aggT[:],
                                lhsT=emb8_t[:, b * D1:(b + 1) * D1],
                                rhs=st[:, (j * 8 + b) * W:(j * 8 + b + 1) * W],
                                start=(b == 0), stop=(b == 7),
                            )
                        h1 = hp.tile([D1, W], bf16, tag="h1")
                        nc.vector.tensor_tensor(
                            out=h1[:], in0=aggT[:], in1=dbt[:, j * W:(j + 1) * W],
                            op=mybir.AluOpType.mult)
                        h1b = hp.tile([D1, W], bf16, tag="h1b")
                        nc.scalar.activation(h1b[:], h1[:], Relu,
                                             bias=b1c_t[:, 0:1], scale=1.0)
                        h1d = hd_all[:, g * W:(g + 1) * W]
                        nc.vector.tensor_tensor(
                            out=h1d, in0=h1b[:], in1=dbt[:, j * W:(j + 1) * W],
                            op=mybir.AluOpType.mult)
                        # transpose to [W, D1] for the slot-major stage table
                        htps = ps2.tile([W, D1], f32, space="PSUM", tag="htps")
                        nc.tensor.matmul(out=htps[:], lhsT=h1d,
                                         rhs=ident_t[:], start=True, stop=True)
                        nc.scalar.activation(hsb[:, j * D1:(j + 1) * D1],
                                             htps[:], Copy, scale=1.0)
                    nc.scalar.dma_start(
                        out=h1_stage[blk * SB * W:(blk + 1) * SB * W, :]
                        .rearrange("(s w) d -> w s d", w=W),
                        in_=hsb[:].rearrange("p (s d) -> p s d", d=D1))

                nc.gpsimd.collective_compute(
                    "AllGather", mybir.AluOpType.bypass, replica_groups=RG,
                    ins=[h1_stage[:]], outs=[h1_full[:]],
                )

                # ---- layer 2: agg2 = segsum(h1d[src]); out = dis*(agg2@W2)+b2
                for b in range(NBATCH):
                    gbs = GBS[b]
                    p0 = int(POS0[b])
                    gatA = gat2p.tile([128, NA[b] * D1], bf16, tag="g2A")
                    batch_gather(gatA, gidxa_t, aoff[b], NA[b],
                                 h1_full[BASE_A:, :], 2 * b)
                    gatB = gat2p.tile([128, NB_[b] * D1], bf16, tag="g2B")
                    batch_gather(gatB, gidxb_t, boff[b], NB_[b],
                                 h1_full[BASE_B:, :], 2 * b + 1)

                    ob = op.tile([128, gbs * D2], f32, tag="ob")
                    for j in range(gbs):
                        p = p0 + j
                        loA, hiA = spanA[b][j]
                        loB, hiB = spanB[b][j]
                        nch = wA[p] + wB[p]
                        d0 = doff[p]
                        oh = ohp.tile([128, CMX * W], bf16, tag="oh")
                        nc.vector.tensor_tensor(
                            out=oh[:, :nch * W].rearrange(
                                "p (c w) -> p c w", w=W),
                            in0=iotac_t[:, :nch * W].rearrange(
                                "p (c w) -> p c w", w=W),
                            in1=dstl_t[:, d0:d0 + nch][:, :, None
                                ].to_broadcast([128, nch, W]),
                            op=mybir.AluOpType.is_equal,
                        )
                        aggH = ps1.tile([D1, W], f32, space="PSUM", tag="aggT")
                        # self-loop diagonal: aggH += h1d of this group
                        nc.tensor.matmul(
                            out=aggH[:], lhsT=ident_t[:],
                            rhs=hd_all[:, p * W:(p + 1) * W],
                            start=True, stop=False,
                        )
                        nmm = (hiA - loA) + (hiB - loB)
                        kk = 0
                        for c in range(loA, hiA):
                            nc.tensor.matmul(
                                out=aggH[:],
                                lhsT=gatA[:, c * D1:(c + 1) * D1],
                                rhs=oh[:, (c - loA) * W:(c - loA + 1) * W],
                                start=False, stop=(kk == nmm - 1),
                            )
                            kk += 1
                        for c in range(loB, hiB):
                            nc.tensor.matmul(
                                out=aggH[:],
                                lhsT=gatB[:, c * D1:(c + 1) * D1],
                                rhs=oh[:, (wA[p] + c - loB) * W:
                                       (wA[p] + c - loB + 1) * W],
                                start=False, stop=(kk == nmm - 1),
                            )
                            kk += 1
                        hT = hp.tile([D1, W], bf16, tag="hT")
                        nc.scalar.activation(hT[:], aggH[:], Copy, scale=1.0)
                        ops = ps2.tile([W, D2], f32, space="PSUM", tag="ops")
                        nc.tensor.matmul(out=ops[:], lhsT=hT[:], rhs=w2_t[:],
                                         start=True, stop=True)
                        o1 = op.tile([W, D2], f32, tag="o1")
                        nc.scalar.activation(o1[:], ops[:], Copy,
                                             scale=discols_t[:, p:p + 1])
                        nc.vector.tensor_tensor(
                            out=ob[:, j * D2:(j + 1) * D2], in0=o1[:],
                            in1=b2r_t[:], op=mybir.AluOpType.add)
                    nc.scalar.dma_start(
                        out=out_ext[p0 * W:(p0 + gbs) * W, :]
                        .rearrange("(s w) d -> w s d", w=W),
                        in_=ob[:].rearrange("p (s d) -> p s d", d=D2))

    nc.compile()
    return nc


def kernel(x_node_types, edge_index, emb, W1, b1, W2, b2):
    from concourse.bass_utils import run_bass_kernel_spmd

    pre = preprocess(x_node_types, edge_index, emb, W1, b1, W2, b2)
    nc = build_program(pre["meta"])

    w2_bf = pre["w2"].astype(BF16)
    ident = np.eye(128, dtype=BF16)
    emb8_bf = pre["emb8"].astype(BF16)
    iotac_bf = pre["iotac"].astype(BF16)
    in_maps = []
    for c in range(NCORE):
        in_maps.append({
            "emb8": emb8_bf, "w2": w2_bf, "ident": ident,
            "iotac": iotac_bf,
            "s": pre["s_up"][c].astype(BF16),
            "gidxa": pre["gidxa_flat"][c], "gidxb": pre["gidxb_flat"][c],
            "dstlpc": pre["dstl_pc"][c].astype(BF16),
            "disb": pre["disb"][c].astype(BF16),
            "discols": pre["dis_cols"][c],
            "b1c": pre["b1c"], "b2r": pre["b2r"],
        })

    trace = bool(int(os.environ.get("BASS_KERNEL_TRACE", "0")))
    res = run_bass_kernel_spmd(nc, in_maps, list(range(NCORE)), trace=trace)
    if trace and res.exec_time_ns is not None:
        print(f"HW exec time: {res.exec_time_ns} ns")

    out = np.zeros((N_NODES, D2), np.float32)
    s2n = pre["slot2node"]
    for c in range(NCORE):
        valid = s2n[c] >= 0
        out[s2n[c, valid]] = res.results[c]["out"][valid]
    return out
